# revision 11
# baseline (speedup 1.0000x reference)
"""Trainium2 Bass kernel for BaseDependentAttentionLayer (GNN message passing).

Strategy (8 NeuronCores, SPMD, no collectives):
  - Nodes sharded by origin: core c owns nodes [c*12500, (c+1)*12500).
  - Every core recomputes the bf16 k|v table for ALL nodes (cheap on PE,
    hides under gather DMA) and q for its own nodes; tables live in DRAM.
  - Edges sharded by origin core, bucketed by (dest-chunk, origin-block),
    padded to 128-edge tiles; per-edge k|v and q fetched with dma_gather
    (int16 indices -> kv table split into 4 chunks of 25088 rows).
  - Segment softmax runs without the max-subtraction pass (shift-invariant;
    values bounded here), so attention reduces to two segment sums that are
    computed with per-tile 0/1 selection-matrix matmuls accumulating into a
    per-128-node-block PSUM tile.  Epilogue (divide, Wo, LayerNorm, residual)
    is fused per block.
"""

import sys

sys.path.insert(0, "/opt/trn_rl_repo")

import numpy as np
import ml_dtypes

import concourse.bass as bass
import concourse.bacc as bacc
import concourse.mybir as mybir
from concourse.tile import TileContext
from concourse.bass_utils import run_bass_kernel_spmd

N = 100000
E = 1600000
D = 64
H = 4
HD = 16
NCORES = 8
NOWN = 12500            # nodes owned per core
NBLK = 98               # 128-node blocks per core
NB = NBLK * 128         # 12544 padded own nodes
NT = 100352             # padded global table rows (= 4 * 25088)
CH = 4                  # dest chunks (int16 gather index limit)
CHROWS = NT // CH       # 25088
GBLK = 3                # node blocks per super-group
LN_EPS = 1e-5
PAD_OID = 200.0         # origin-id sentinel for pad slots (matches no node)

F32 = mybir.dt.float32
BF16 = mybir.dt.bfloat16
I16 = mybir.dt.int16
BF16_NP = ml_dtypes.bfloat16


def _build_structure(origins, dests):
    """Global (core-independent) stream structure + per-core slot data.

    origins/dests: full [E] int arrays.
    Returns (struct, per_core) where struct is identical for all cores.
    """
    owner = origins // NOWN
    per_core_raw = []
    tcnt = np.zeros((NCORES, CH * NBLK), np.int64)
    for c in range(NCORES):
        m = owner == c
        o = (origins[m] - c * NOWN).astype(np.int32)
        d = dests[m].astype(np.int32)
        eids = np.nonzero(m)[0]
        blk = o >> 7
        chunk = d // CHROWS
        cell = chunk * NBLK + blk
        order = np.argsort(cell, kind="stable")
        o, d, eids, cell = o[order], d[order], eids[order], cell[order]
        cnt = np.bincount(cell, minlength=CH * NBLK)
        tcnt[c] = (cnt + 127) // 128
        per_core_raw.append((o, d, eids, cnt))
    cell_tiles = tcnt.max(0)          # [CH*NBLK] tiles per cell, all cores
    cell_tiles = np.maximum(cell_tiles, 1)

    sg_blocks = [list(range(s, min(s + GBLK, NBLK))) for s in range(0, NBLK, GBLK)]
    # stream order: sg -> chunk -> block
    sgs = []
    tile_block = []     # per global tile: block id
    tile_sg = []
    cell_tile_off = np.zeros(CH * NBLK, np.int64)
    t_off = 0
    for blocks in sg_blocks:
        T_c = []
        sg_tiles = []
        for ch in range(CH):
            tc = 0
            for b in blocks:
                cell = ch * NBLK + b
                nt = int(cell_tiles[cell])
                cell_tile_off[cell] = t_off
                tile_block.extend([b] * nt)
                sg_tiles.extend([b] * nt)
                tile_sg.extend([len(sgs)] * nt)
                t_off += nt
                tc += nt
            T_c.append(tc)
        # start/stop flags within the sg
        first = {}
        last = {}
        for i, b in enumerate(sg_tiles):
            if b not in first:
                first[b] = i
            last[b] = i
        sgs.append({
            "blocks": blocks,
            "T_c": T_c,
            "T": sum(T_c),
            "tile_blocks": sg_tiles,
            "first": first,
            "last": last,
            "tile_off": t_off - sum(T_c),
        })
    S_tiles = t_off
    struct = {
        "sgs": sgs,
        "S_tiles": S_tiles,
        "cell_tiles": cell_tiles,
        "cell_tile_off": cell_tile_off,
    }
    return struct, per_core_raw


def _per_core_arrays(struct, core_raw, edge_weights):
    """Build the per-core DRAM-side slot arrays."""
    o, d, eids, cnt = core_raw
    S_tiles = struct["S_tiles"]
    S = S_tiles * 128
    cell_tiles = struct["cell_tiles"]
    cell_tile_off = struct["cell_tile_off"]

    oid = np.full(S, PAD_OID, np.float32)
    kvi = np.zeros(S, np.int16)
    qi = np.zeros(S, np.int16)
    ew4 = np.zeros((S, H), np.float32)

    # place each cell's edges at its slot range
    cell_edge_off = np.zeros(CH * NBLK + 1, np.int64)
    np.cumsum(cnt, out=cell_edge_off[1:])
    for cell in range(CH * NBLK):
        n = int(cnt[cell])
        if n == 0:
            continue
        e0 = cell_edge_off[cell]
        s0 = cell_tile_off[cell] * 128
        ch = cell // NBLK
        sl = slice(s0, s0 + n)
        el = slice(e0, e0 + n)
        oid[sl] = (o[el] & 127).astype(np.float32)
        kvi[sl] = (d[el] - ch * CHROWS).astype(np.int16)
        qi[sl] = o[el].astype(np.int16)
        ew4[sl] = edge_weights[eids[el]] * (HD ** -0.5)

    # tile-major [128, S_tiles(,H)] views (partition = slot % 128)
    oid_t = np.ascontiguousarray(
        oid.reshape(S_tiles, 128).T).astype(BF16_NP)
    ew_t = np.ascontiguousarray(ew4.reshape(S_tiles, 128, H).transpose(1, 0, 2))

    # wrapped int16 index arrays: per-run [16, len/16] replicated to 128 parts
    def wrap(run_vals):
        w = run_vals.reshape(-1, 16).T          # [16, len/16]
        return np.tile(w, (8, 1))               # [128, len/16]

    kvw = np.zeros((128, S_tiles * 8), np.int16)
    qw = np.zeros((128, S_tiles * 8), np.int16)
    for sg in struct["sgs"]:
        t0 = sg["tile_off"]
        qw[:, t0 * 8:(t0 + sg["T"]) * 8] = wrap(qi[t0 * 128:(t0 + sg["T"]) * 128])
        off = t0
        for ch in range(CH):
            tc = sg["T_c"][ch]
            kvw[:, off * 8:(off + tc) * 8] = wrap(kvi[off * 128:(off + tc) * 128])
            off += tc
    return {"oid": oid_t, "ew": ew_t, "kvidx": kvw, "qidx": qw}


def _build_graph(struct, sg_limit=None, dump_tab=False, no_gather=False,
                 gather_only=False):
    nc = bacc.Bacc()
    S_tiles = struct["S_tiles"]

    xT = nc.declare_dram_parameter("xT", [D + 1, NT], F32, isOutput=False)
    xTo = nc.declare_dram_parameter("xTo", [D + 1, NB], F32, isOutput=False)
    wkv = nc.declare_dram_parameter("wkv", [D + 1, 2 * D], F32, isOutput=False)
    wq = nc.declare_dram_parameter("wq", [D + 1, 2 * D], F32, isOutput=False)
    wot = nc.declare_dram_parameter("wot", [D, D], F32, isOutput=False)
    boc = nc.declare_dram_parameter("boc", [128, D], F32, isOutput=False)
    gam = nc.declare_dram_parameter("gam", [128, D], F32, isOutput=False)
    iot = nc.declare_dram_parameter("iot", [128, 128], BF16, isOutput=False)
    idn = nc.declare_dram_parameter("idn", [128, 128], F32, isOutput=False)
    xpb = nc.declare_dram_parameter("xpb", [NB, D], F32, isOutput=False)
    oid = nc.declare_dram_parameter("oid", [128, S_tiles], BF16, isOutput=False)
    ewp = nc.declare_dram_parameter("ewp", [128, S_tiles, H], F32, isOutput=False)
    kvx = nc.declare_dram_parameter("kvx", [128, S_tiles * 8], I16, isOutput=False)
    qx = nc.declare_dram_parameter("qx", [128, S_tiles * 8], I16, isOutput=False)
    out = nc.declare_dram_parameter("out", [NB, D], F32, isOutput=True)

    kv_tab = nc.dram_tensor("kv_tab", [NT, 2 * D], BF16)
    q_tab = nc.dram_tensor("q_tab", [NB, 2 * D], BF16)
    if dump_tab:
        kv_dump = nc.declare_dram_parameter(
            "kv_dump", [1024, 2 * D], BF16, isOutput=True)
        q_dump = nc.declare_dram_parameter(
            "q_dump", [1024, 2 * D], BF16, isOutput=True)
    if gather_only:
        T0 = struct["sgs"][0]["T"]
        g_dump = nc.declare_dram_parameter(
            "g_dump", [128, T0, 2 * D], BF16, isOutput=True)
        g_dump2 = nc.declare_dram_parameter(
            "g_dump2", [128, T0, 2 * D], BF16, isOutput=True)

    with TileContext(nc) as tc:
        with tc.tile_pool(name="const", bufs=1) as cp:
            wkv_t = cp.tile([D + 1, 2 * D], F32)
            nc.sync.dma_start(out=wkv_t[:], in_=wkv[:])
            wq_t = cp.tile([D + 1, 2 * D], F32)
            nc.sync.dma_start(out=wq_t[:], in_=wq[:])
            wot_f = cp.tile([D, D], F32)
            nc.sync.dma_start(out=wot_f[:], in_=wot[:])
            wot_t = cp.tile([D, D], BF16)
            nc.vector.tensor_copy(wot_t[:], wot_f[:])
            boc_t = cp.tile([128, D], F32)
            nc.sync.dma_start(out=boc_t[:], in_=boc[:])
            gam_t = cp.tile([128, D], F32)
            nc.sync.dma_start(out=gam_t[:], in_=gam[:])
            iot_t = cp.tile([128, 128], BF16)
            nc.sync.dma_start(out=iot_t[:], in_=iot[:])
            idn_t = cp.tile([128, 128], F32)
            nc.sync.dma_start(out=idn_t[:], in_=idn[:])

            # ---- phase 1: build kv table (all nodes) + q table (own nodes)
            with (
                tc.tile_pool(name="p1sb", bufs=3) as p1,
                tc.tile_pool(name="p1ps", bufs=2, space="PSUM") as p1p,
            ):
                def qkv_slab(src, wt, tab, s, ntile):
                    xs = p1.tile([D + 1, ntile * 128], F32, tag="xs")
                    nc.sync.dma_start(
                        out=xs[:], in_=src[:, s * 1024:s * 1024 + ntile * 128])
                    ps = p1p.tile([128, ntile * 128], F32, tag="ps")
                    for j in range(ntile):
                        nc.tensor.matmul(
                            out=ps[:, j * 128:(j + 1) * 128],
                            lhsT=xs[:, j * 128:(j + 1) * 128],
                            rhs=wt[:],
                            start=True, stop=True)
                    sb = p1.tile([128, ntile, 128], BF16, tag="sb")
                    nc.vector.tensor_copy(
                        sb[:].rearrange("p a d -> p (a d)"),
                        ps[:])
                    for j in range(ntile):
                        nc.sync.dma_start(
                            out=tab[s * 1024 + j * 128:s * 1024 + (j + 1) * 128, :],
                            in_=sb[:, j, :])

                for s in range(NT // 1024):                    # 98 slabs
                    qkv_slab(xT, wkv_t, kv_tab, s, 8)
                for s in range(NB // 1024):                    # 12 slabs
                    qkv_slab(xTo, wq_t, q_tab, s, 8)
                qkv_slab(xTo, wq_t, q_tab, NB // 1024, (NB % 1024) // 128)

            if dump_tab:
                nc.sync.dma_start(out=kv_dump[:], in_=kv_tab[0:1024, :])
                nc.sync.dma_start(out=q_dump[:], in_=q_tab[0:1024, :])
            sgs_run = struct["sgs"] if sg_limit is None else struct["sgs"][:sg_limit]

            # ---- phase 2: edge processing per super-group
            with (
                tc.tile_pool(name="gat", bufs=2) as gp,
                tc.tile_pool(name="met", bufs=2) as mp,
                tc.tile_pool(name="wrk", bufs=2) as wp,
                tc.tile_pool(name="pst", bufs=2) as pp,
                tc.tile_pool(name="bps", bufs=GBLK + 1, space="PSUM") as bp,
                tc.tile_pool(name="tps", bufs=2, space="PSUM") as tp,
                tc.tile_pool(name="ops", bufs=2, space="PSUM") as op,
            ):
                for sg in sgs_run:
                    T = sg["T"]
                    t0 = sg["tile_off"]
                    kvi_t = mp.tile([128, T * 8], I16, tag="kvi")
                    nc.sync.dma_start(out=kvi_t[:], in_=kvx[:, t0 * 8:(t0 + T) * 8])
                    qi_t = mp.tile([128, T * 8], I16, tag="qi")
                    nc.sync.dma_start(out=qi_t[:], in_=qx[:, t0 * 8:(t0 + T) * 8])
                    oid_t = mp.tile([128, T], BF16, tag="oid")
                    nc.sync.dma_start(out=oid_t[:], in_=oid[:, t0:t0 + T])
                    ew_t = mp.tile([128, T, H], F32, tag="ew")
                    nc.sync.dma_start(out=ew_t[:], in_=ewp[:, t0:t0 + T, :])

                    kvg = gp.tile([128, T, 2 * D], BF16, tag="kvg")
                    qg = gp.tile([128, T, 2 * D], BF16, tag="qg")
                    if no_gather:
                        nc.gpsimd.memset(kvg[:], 1.0)
                        nc.gpsimd.memset(qg[:], 1.0)
                    else:
                        off = 0
                        for ch in range(CH):
                            tcn = sg["T_c"][ch]
                            nc.gpsimd.dma_gather(
                                out_ap=kvg[:, off:off + tcn, :],
                                in_ap=kv_tab[ch * CHROWS:(ch + 1) * CHROWS, :],
                                idxs_ap=kvi_t[:, off * 8:(off + tcn) * 8],
                                num_idxs=tcn * 128,
                                num_idxs_reg=tcn * 128,
                                elem_size=2 * D,
                                single_packet=False)
                            off += tcn
                        nc.gpsimd.dma_gather(
                            out_ap=qg[:],
                            in_ap=q_tab[:],
                            idxs_ap=qi_t[:],
                            num_idxs=T * 128,
                            num_idxs_reg=T * 128,
                            elem_size=2 * D,
                            single_packet=False)
                    if gather_only:
                        nc.sync.dma_start(out=g_dump[:], in_=kvg[:])
                        nc.sync.dma_start(out=g_dump2[:], in_=qg[:])
                        continue

                    qk = wp.tile([128, T, D], BF16, tag="qk")
                    nc.vector.tensor_tensor(
                        out=qk[:], in0=qg[:, :, 0:D], in1=kvg[:, :, 0:D],
                        op=mybir.AluOpType.mult)
                    sc = wp.tile([128, T, H], F32, tag="sc")
                    nc.vector.tensor_reduce(
                        out=sc[:],
                        in_=qk[:].rearrange("p t (h d) -> p t h d", h=H),
                        axis=mybir.AxisListType.X, op=mybir.AluOpType.add)
                    ws = wp.tile([128, T, H], F32, tag="ws")
                    nc.vector.tensor_tensor(
                        out=ws[:], in0=sc[:], in1=ew_t[:],
                        op=mybir.AluOpType.mult)
                    ex = wp.tile([128, T, H], BF16, tag="ex")
                    nc.scalar.activation(
                        out=ex[:], in_=ws[:],
                        func=mybir.ActivationFunctionType.Exp)
                    ctb = wp.tile([128, T, D + H], BF16, tag="ctb")
                    nc.vector.tensor_copy(ctb[:, :, D:D + H], ex[:])
                    nc.vector.tensor_tensor(
                        out=ctb[:, :, 0:D].rearrange("p t (h d) -> p t h d", h=H),
                        in0=kvg[:, :, D:2 * D].rearrange("p t (h d) -> p t h d", h=H),
                        in1=ex[:].rearrange("p t (h o) -> p t h o", o=1)
                            .to_broadcast([128, T, H, HD]),
                        op=mybir.AluOpType.mult)
                    sel = wp.tile([128, T, 128], BF16, tag="sel")
                    nc.vector.tensor_tensor(
                        out=sel[:],
                        in0=oid_t[:].rearrange("p (t o) -> p t o", o=1)
                            .to_broadcast([128, T, 128]),
                        in1=iot_t[:].rearrange("p (o n) -> p o n", o=1)
                            .to_broadcast([128, T, 128]),
                        op=mybir.AluOpType.is_equal)

                    psums = {}
                    for b in sg["blocks"]:
                        psums[b] = bp.tile([128, D + H], F32, tag="bps",
                                           name=f"bps{b}")
                    for i, b in enumerate(sg["tile_blocks"]):
                        nc.tensor.matmul(
                            out=psums[b][:],
                            lhsT=sel[:, i, :],
                            rhs=ctb[:, i, :],
                            start=(sg["first"][b] == i),
                            stop=(sg["last"][b] == i))

                    for b in sg["blocks"]:
                        ps = psums[b]
                        z = pp.tile([128, H], F32, tag="z")
                        nc.vector.tensor_scalar_add(z[:], ps[:, D:D + H], 1e-16)
                        zr = pp.tile([128, H], F32, tag="zr")
                        nc.vector.reciprocal(zr[:], z[:])
                        vals = pp.tile([128, D], F32, tag="vals")
                        nc.vector.tensor_tensor(
                            out=vals[:].rearrange("p (h d) -> p h d", h=H),
                            in0=ps[:, 0:D].rearrange("p (h d) -> p h d", h=H),
                            in1=zr[:].rearrange("p (h o) -> p h o", o=1)
                                .to_broadcast([128, H, HD]),
                            op=mybir.AluOpType.mult)
                        pt = tp.tile([D, 128], F32, tag="pt")
                        nc.tensor.transpose(out=pt[:], in_=vals[:], identity=idn_t[:])
                        vT = pp.tile([D, 128], BF16, tag="vT")
                        nc.vector.tensor_copy(vT[:], pt[:])
                        po = op.tile([128, D], F32, tag="po")
                        nc.tensor.matmul(out=po[:], lhsT=vT[:], rhs=wot_t[:],
                                         start=True, stop=True)
                        # LayerNorm + residual
                        s1 = pp.tile([128, 1], F32, tag="s1")
                        nc.vector.tensor_reduce(
                            out=s1[:], in_=po[:],
                            axis=mybir.AxisListType.X, op=mybir.AluOpType.add)
                        nmu = pp.tile([128, 1], F32, tag="nmu")
                        nc.vector.tensor_scalar_mul(nmu[:], s1[:], -1.0 / D)
                        ct = pp.tile([128, D], F32, tag="ct")
                        nc.vector.tensor_tensor(
                            out=ct[:], in0=po[:],
                            in1=nmu[:].to_broadcast([128, D]),
                            op=mybir.AluOpType.add)
                        nc.vector.tensor_tensor(
                            out=ct[:], in0=ct[:], in1=boc_t[:],
                            op=mybir.AluOpType.add)
                        sq = pp.tile([128, D], F32, tag="sq")
                        nc.vector.tensor_tensor(
                            out=sq[:], in0=ct[:], in1=ct[:],
                            op=mybir.AluOpType.mult)
                        v1 = pp.tile([128, 1], F32, tag="v1")
                        nc.vector.tensor_reduce(
                            out=v1[:], in_=sq[:],
                            axis=mybir.AxisListType.X, op=mybir.AluOpType.add)
                        nc.vector.tensor_scalar(
                            out=v1[:], in0=v1[:],
                            scalar1=1.0 / D, scalar2=LN_EPS,
                            op0=mybir.AluOpType.mult, op1=mybir.AluOpType.add)
                        nc.vector.reciprocal(v1[:], v1[:])
                        rstd = pp.tile([128, 1], F32, tag="rstd")
                        nc.scalar.sqrt(rstd[:], v1[:])
                        ln = pp.tile([128, D], F32, tag="ln")
                        nc.scalar.activation(
                            out=ln[:], in_=ct[:],
                            func=mybir.ActivationFunctionType.Copy,
                            scale=rstd[:, 0:1])
                        xb = pp.tile([128, D], F32, tag="xb")
                        nc.sync.dma_start(
                            out=xb[:], in_=xpb[b * 128:(b + 1) * 128, :])
                        ot = pp.tile([128, D], F32, tag="ot")
                        nc.vector.tensor_tensor(
                            out=ot[:], in0=ln[:], in1=gam_t[:],
                            op=mybir.AluOpType.mult)
                        nc.vector.tensor_tensor(
                            out=ot[:], in0=ot[:], in1=xb[:],
                            op=mybir.AluOpType.add)
                        nc.sync.dma_start(
                            out=out[b * 128:(b + 1) * 128, :], in_=ot[:])
    return nc


def kernel(x, edge_index, edge_weights, Wq, bq, Wk, bk, Wv, bv, Wo, bo,
           gamma, beta):
    x = np.asarray(x, np.float32)
    edge_index = np.asarray(edge_index)
    edge_weights = np.asarray(edge_weights, np.float32)
    origins = np.asarray(edge_index[0], np.int64)
    dests = np.asarray(edge_index[1], np.int64)

    struct, per_core_raw = _build_structure(origins, dests)
    nc = _build_graph(struct)
    nc.finalize()

    # shared (replicated) host arrays
    xT = np.zeros((D + 1, NT), np.float32)
    xT[:D, :N] = x.T
    xT[D] = 1.0
    wkv = np.zeros((D + 1, 2 * D), np.float32)
    wkv[:D, :D] = np.asarray(Wk, np.float32).T
    wkv[:D, D:] = np.asarray(Wv, np.float32).T
    wkv[D, :D] = np.asarray(bk, np.float32)
    wkv[D, D:] = np.asarray(bv, np.float32)
    wq = np.zeros((D + 1, 2 * D), np.float32)
    wq[:D, :D] = np.asarray(Wq, np.float32).T
    wq[D, :D] = np.asarray(bq, np.float32)
    wot = np.ascontiguousarray(np.asarray(Wo, np.float32).T)
    bo = np.asarray(bo, np.float32)
    boc = np.tile((bo - bo.mean())[None, :], (128, 1)).astype(np.float32)
    gam_t = np.tile(np.asarray(gamma, np.float32)[None, :], (128, 1))
    iot = np.tile(np.arange(128, dtype=np.float32)[None, :], (128, 1)
                  ).astype(BF16_NP)
    idn = np.eye(128, dtype=np.float32)

    in_maps = []
    for c in range(NCORES):
        data = _per_core_arrays(struct, per_core_raw[c], edge_weights)
        xTo = np.zeros((D + 1, NB), np.float32)
        xTo[:D, :NOWN] = x[c * NOWN:(c + 1) * NOWN].T
        xTo[D] = 1.0
        xpb = np.zeros((NB, D), np.float32)
        xpb[:NOWN] = x[c * NOWN:(c + 1) * NOWN] + np.asarray(beta, np.float32)
        in_maps.append({
            "xT": xT, "xTo": xTo, "wkv": wkv, "wq": wq, "wot": wot,
            "boc": boc, "gam": gam_t, "iot": iot, "idn": idn, "xpb": xpb,
            "oid": data["oid"], "ewp": data["ew"],
            "kvx": data["kvidx"], "qx": data["qidx"],
        })

    res = run_bass_kernel_spmd(nc, in_maps, core_ids=list(range(NCORES)),
                               trace=TRACE)
    global LAST_RESULT
    LAST_RESULT = res
    outs = [np.asarray(res.results[i]["out"])[:NOWN] for i in range(NCORES)]
    return np.concatenate(outs, axis=0).astype(np.float32)


TRACE = False
LAST_RESULT = None


# revision 15
# speedup vs baseline: 1.8191x; 1.8191x over previous
"""Trainium2 Bass kernel for BaseDependentAttentionLayer (GNN message passing).

Strategy (8 NeuronCores, SPMD, no collectives):
  - Nodes sharded by origin: core c owns nodes [c*12500, (c+1)*12500).
  - Every core recomputes the bf16 k|v table for ALL nodes (cheap on PE,
    hides under gather DMA) and q for its own nodes; tables live in DRAM.
  - Edges sharded by origin core, bucketed by (dest-chunk, origin-block),
    padded to 128-edge tiles; per-edge k|v and q fetched with dma_gather
    (int16 indices -> kv table split into 4 chunks of 25088 rows).
  - Segment softmax runs without the max-subtraction pass (shift-invariant;
    values bounded here), so attention reduces to two segment sums that are
    computed with per-tile 0/1 selection-matrix matmuls accumulating into a
    per-128-node-block PSUM tile.  Epilogue (divide, Wo, LayerNorm, residual)
    is fused per block.
"""

import sys

sys.path.insert(0, "/opt/trn_rl_repo")

import numpy as np
import ml_dtypes

import concourse.bass as bass
import concourse.bacc as bacc
import concourse.mybir as mybir
from concourse.tile import TileContext
from concourse.bass_utils import run_bass_kernel_spmd

N = 100000
E = 1600000
D = 64
H = 4
HD = 16
NCORES = 8
NOWN = 12500            # nodes owned per core
NBLK = 98               # 128-node blocks per core
NB = NBLK * 128         # 12544 padded own nodes
NT = 100352             # padded global table rows (= 4 * 25088)
CH = 4                  # dest chunks (int16 gather index limit)
CHROWS = NT // CH       # 25088
GBLK = 3                # node blocks per super-group
LN_EPS = 1e-5
PAD_OID = 200.0         # origin-id sentinel for pad slots (matches no node)

F32 = mybir.dt.float32
BF16 = mybir.dt.bfloat16
I16 = mybir.dt.int16
BF16_NP = ml_dtypes.bfloat16


def _build_structure(origins, dests):
    """Global (core-independent) stream structure + per-core slot data.

    origins/dests: full [E] int arrays.
    Returns (struct, per_core) where struct is identical for all cores.
    """
    owner = origins // NOWN
    per_core_raw = []
    tcnt = np.zeros((NCORES, CH * NBLK), np.int64)
    for c in range(NCORES):
        m = owner == c
        o = (origins[m] - c * NOWN).astype(np.int32)
        d = dests[m].astype(np.int32)
        eids = np.nonzero(m)[0]
        blk = o >> 7
        chunk = d // CHROWS
        cell = chunk * NBLK + blk
        order = np.argsort(cell, kind="stable")
        o, d, eids, cell = o[order], d[order], eids[order], cell[order]
        cnt = np.bincount(cell, minlength=CH * NBLK)
        tcnt[c] = (cnt + 127) // 128
        per_core_raw.append((o, d, eids, cnt))
    cell_tiles = tcnt.max(0)          # [CH*NBLK] tiles per cell, all cores
    cell_tiles = np.maximum(cell_tiles, 1)

    sg_blocks = [list(range(s, min(s + GBLK, NBLK))) for s in range(0, NBLK, GBLK)]
    # stream order: sg -> chunk -> block
    sgs = []
    tile_block = []     # per global tile: block id
    tile_sg = []
    cell_tile_off = np.zeros(CH * NBLK, np.int64)
    t_off = 0
    for blocks in sg_blocks:
        T_c = []
        sg_tiles = []
        for ch in range(CH):
            tc = 0
            for b in blocks:
                cell = ch * NBLK + b
                nt = int(cell_tiles[cell])
                cell_tile_off[cell] = t_off
                tile_block.extend([b] * nt)
                sg_tiles.extend([b] * nt)
                tile_sg.extend([len(sgs)] * nt)
                t_off += nt
                tc += nt
            T_c.append(tc)
        # start/stop flags within the sg
        first = {}
        last = {}
        for i, b in enumerate(sg_tiles):
            if b not in first:
                first[b] = i
            last[b] = i
        sgs.append({
            "blocks": blocks,
            "T_c": T_c,
            "T": sum(T_c),
            "tile_blocks": sg_tiles,
            "first": first,
            "last": last,
            "tile_off": t_off - sum(T_c),
        })
    S_tiles = t_off
    struct = {
        "sgs": sgs,
        "S_tiles": S_tiles,
        "cell_tiles": cell_tiles,
        "cell_tile_off": cell_tile_off,
    }
    return struct, per_core_raw


def _per_core_arrays(struct, core_raw, edge_weights):
    """Build the per-core DRAM-side slot arrays."""
    o, d, eids, cnt = core_raw
    S_tiles = struct["S_tiles"]
    S = S_tiles * 128
    cell_tiles = struct["cell_tiles"]
    cell_tile_off = struct["cell_tile_off"]

    oid = np.full(S, PAD_OID, np.float32)
    kvi = np.zeros(S, np.int16)
    qi = np.zeros(S, np.int16)
    ew4 = np.zeros((S, H), np.float32)

    # place each cell's edges at its slot range
    cell_edge_off = np.zeros(CH * NBLK + 1, np.int64)
    np.cumsum(cnt, out=cell_edge_off[1:])
    for cell in range(CH * NBLK):
        n = int(cnt[cell])
        if n == 0:
            continue
        e0 = cell_edge_off[cell]
        s0 = cell_tile_off[cell] * 128
        ch = cell // NBLK
        sl = slice(s0, s0 + n)
        el = slice(e0, e0 + n)
        oid[sl] = (o[el] & 127).astype(np.float32)
        kvi[sl] = (d[el] - ch * CHROWS).astype(np.int16)
        qi[sl] = o[el].astype(np.int16)
        ew4[sl] = edge_weights[eids[el]] * (HD ** -0.5)

    # tile-major [128, S_tiles(,H)] views (partition = slot % 128)
    oid_t = np.ascontiguousarray(
        oid.reshape(S_tiles, 128).T).astype(BF16_NP)
    ew_t = np.ascontiguousarray(ew4.reshape(S_tiles, 128, H).transpose(1, 0, 2))

    # wrapped int16 index arrays: per-run [16, len/16] replicated to 128 parts
    def wrap(run_vals):
        w = run_vals.reshape(-1, 16).T          # [16, len/16]
        return np.tile(w, (8, 1))               # [128, len/16]

    kvw = np.zeros((128, S_tiles * 8), np.int16)
    qw = np.zeros((128, S_tiles * 8), np.int16)
    for sg in struct["sgs"]:
        t0 = sg["tile_off"]
        qw[:, t0 * 8:(t0 + sg["T"]) * 8] = wrap(qi[t0 * 128:(t0 + sg["T"]) * 128])
        off = t0
        for ch in range(CH):
            tc = sg["T_c"][ch]
            kvw[:, off * 8:(off + tc) * 8] = wrap(kvi[off * 128:(off + tc) * 128])
            off += tc
    return {"oid": oid_t, "ew": ew_t, "kvidx": kvw, "qidx": qw}


def _build_graph(struct, sg_limit=None, dump_tab=False, no_gather=False,
                 gather_only=False):
    nc = bacc.Bacc()
    S_tiles = struct["S_tiles"]

    xT = nc.declare_dram_parameter("xT", [D + 1, NT], BF16, isOutput=False)
    xTo = nc.declare_dram_parameter("xTo", [D + 1, NB], BF16, isOutput=False)
    wkv = nc.declare_dram_parameter("wkv", [D + 1, 2 * D], BF16, isOutput=False)
    wq = nc.declare_dram_parameter("wq", [D + 1, 2 * D], BF16, isOutput=False)
    wot = nc.declare_dram_parameter("wot", [D, D], F32, isOutput=False)
    boc = nc.declare_dram_parameter("boc", [128, D], F32, isOutput=False)
    gam = nc.declare_dram_parameter("gam", [128, D], F32, isOutput=False)
    iot = nc.declare_dram_parameter("iot", [128, 128], BF16, isOutput=False)
    idn = nc.declare_dram_parameter("idn", [128, 128], F32, isOutput=False)
    xpb = nc.declare_dram_parameter("xpb", [NB, D], F32, isOutput=False)
    oid = nc.declare_dram_parameter("oid", [128, S_tiles], BF16, isOutput=False)
    ewp = nc.declare_dram_parameter("ewp", [128, S_tiles, H], F32, isOutput=False)
    kvx = nc.declare_dram_parameter("kvx", [128, S_tiles * 8], I16, isOutput=False)
    qx = nc.declare_dram_parameter("qx", [128, S_tiles * 8], I16, isOutput=False)
    out = nc.declare_dram_parameter("out", [NB, D], F32, isOutput=True)

    kv_tab = nc.dram_tensor("kv_tab", [NT, 2 * D], BF16)
    q_tab = nc.dram_tensor("q_tab", [NB, 2 * D], BF16)
    if dump_tab:
        kv_dump = nc.declare_dram_parameter(
            "kv_dump", [1024, 2 * D], BF16, isOutput=True)
        q_dump = nc.declare_dram_parameter(
            "q_dump", [1024, 2 * D], BF16, isOutput=True)
    if gather_only:
        T0 = struct["sgs"][0]["T"]
        g_dump = nc.declare_dram_parameter(
            "g_dump", [128, T0, 2 * D], BF16, isOutput=True)
        g_dump2 = nc.declare_dram_parameter(
            "g_dump2", [128, T0, 2 * D], BF16, isOutput=True)

    with TileContext(nc) as tc:
        with tc.tile_pool(name="const", bufs=1) as cp:
            wkv_t = cp.tile([D + 1, 2 * D], BF16)
            nc.sync.dma_start(out=wkv_t[:], in_=wkv[:])
            wq_t = cp.tile([D + 1, 2 * D], BF16)
            nc.sync.dma_start(out=wq_t[:], in_=wq[:])
            wot_f = cp.tile([D, D], F32)
            nc.sync.dma_start(out=wot_f[:], in_=wot[:])
            wot_t = cp.tile([D, D], BF16)
            nc.vector.tensor_copy(wot_t[:], wot_f[:])
            boc_t = cp.tile([128, D], F32)
            nc.sync.dma_start(out=boc_t[:], in_=boc[:])
            gam_t = cp.tile([128, D], F32)
            nc.sync.dma_start(out=gam_t[:], in_=gam[:])
            iot_t = cp.tile([128, 128], BF16)
            nc.sync.dma_start(out=iot_t[:], in_=iot[:])
            idn_t = cp.tile([128, 128], F32)
            nc.sync.dma_start(out=idn_t[:], in_=idn[:])

            # ---- phase 1: build kv table (all nodes) + q table (own nodes)
            SLAB = 16                                          # tiles per slab
            with (
                tc.tile_pool(name="p1sb", bufs=3) as p1,
                tc.tile_pool(name="p1ps", bufs=2, space="PSUM") as p1p,
            ):
                def qkv_slab(src, wt, tab, s, ntile):
                    n0 = s * SLAB * 128
                    xs = p1.tile([D + 1, ntile * 128], BF16, tag="xs")
                    nc.scalar.dma_start(
                        out=xs[:], in_=src[:, n0:n0 + ntile * 128])
                    ps = p1p.tile([128, ntile * 128], F32, tag="ps")
                    for j in range(ntile):
                        nc.tensor.matmul(
                            out=ps[:, j * 128:(j + 1) * 128],
                            lhsT=xs[:, j * 128:(j + 1) * 128],
                            rhs=wt[:],
                            start=True, stop=True)
                    sb = p1.tile([128, ntile, 128], BF16, tag="sb")
                    nc.scalar.copy(
                        sb[:].rearrange("p a d -> p (a d)"),
                        ps[:])
                    nc.sync.dma_start(
                        out=tab[n0:n0 + ntile * 128, :]
                            .rearrange("(a p) d -> p a d", p=128),
                        in_=sb[:])

                for s in range(NT // (SLAB * 128)):            # 49 slabs
                    qkv_slab(xT, wkv_t, kv_tab, s, SLAB)
                nfull_q = NB // (SLAB * 128)                   # 6 slabs
                for s in range(nfull_q):
                    qkv_slab(xTo, wq_t, q_tab, s, SLAB)
                qkv_slab(xTo, wq_t, q_tab, nfull_q,
                         (NB - nfull_q * SLAB * 128) // 128)

            if dump_tab:
                nc.sync.dma_start(out=kv_dump[:], in_=kv_tab[0:1024, :])
                nc.sync.dma_start(out=q_dump[:], in_=q_tab[0:1024, :])
            sgs_run = struct["sgs"] if sg_limit is None else struct["sgs"][:sg_limit]

            # ---- phase 2: edge processing per super-group
            with (
                tc.tile_pool(name="gat", bufs=2) as gp,
                tc.tile_pool(name="met", bufs=2) as mp,
                tc.tile_pool(name="wrk", bufs=2) as wp,
                tc.tile_pool(name="pst", bufs=2) as pp,
                tc.tile_pool(name="bps", bufs=GBLK + 1, space="PSUM") as bp,
                tc.tile_pool(name="tps", bufs=2, space="PSUM") as tp,
                tc.tile_pool(name="ops", bufs=2, space="PSUM") as op,
            ):
                for sg_i, sg in enumerate(sgs_run):
                    T = sg["T"]
                    t0 = sg["tile_off"]
                    kvi_t = mp.tile([128, T * 8], I16, tag="kvi")
                    nc.scalar.dma_start(out=kvi_t[:], in_=kvx[:, t0 * 8:(t0 + T) * 8])
                    qi_t = mp.tile([128, T * 8], I16, tag="qi")
                    nc.scalar.dma_start(out=qi_t[:], in_=qx[:, t0 * 8:(t0 + T) * 8])
                    oid_t = mp.tile([128, T], BF16, tag="oid")
                    nc.sync.dma_start(out=oid_t[:], in_=oid[:, t0:t0 + T])
                    ew_t = mp.tile([128, T, H], F32, tag="ew")
                    nc.sync.dma_start(out=ew_t[:], in_=ewp[:, t0:t0 + T, :])

                    kvg = gp.tile([128, T, 2 * D], BF16, tag="kvg")
                    qg = gp.tile([128, T, 2 * D], BF16, tag="qg")
                    if no_gather:
                        nc.gpsimd.memset(kvg[:], 1.0)
                        nc.gpsimd.memset(qg[:], 1.0)
                    else:
                        off = 0
                        for ch in range(CH):
                            tcn = sg["T_c"][ch]
                            nc.gpsimd.dma_gather(
                                out_ap=kvg[:, off:off + tcn, :],
                                in_ap=kv_tab[ch * CHROWS:(ch + 1) * CHROWS, :],
                                idxs_ap=kvi_t[:, off * 8:(off + tcn) * 8],
                                num_idxs=tcn * 128,
                                num_idxs_reg=tcn * 128,
                                elem_size=2 * D,
                                single_packet=False)
                            off += tcn
                        nc.gpsimd.dma_gather(
                            out_ap=qg[:],
                            in_ap=q_tab[:],
                            idxs_ap=qi_t[:],
                            num_idxs=T * 128,
                            num_idxs_reg=T * 128,
                            elem_size=2 * D,
                            single_packet=False)
                    if gather_only:
                        nc.sync.dma_start(out=g_dump[:], in_=kvg[:])
                        nc.sync.dma_start(out=g_dump2[:], in_=qg[:])
                        continue

                    qk = wp.tile([128, T, D], BF16, tag="qk")
                    nc.vector.tensor_tensor(
                        out=qk[:], in0=qg[:, :, 0:D], in1=kvg[:, :, 0:D],
                        op=mybir.AluOpType.mult)
                    sc = wp.tile([128, T, H], F32, tag="sc")
                    nc.vector.tensor_reduce(
                        out=sc[:],
                        in_=qk[:].rearrange("p t (h d) -> p t h d", h=H),
                        axis=mybir.AxisListType.X, op=mybir.AluOpType.add)
                    ws = wp.tile([128, T, H], F32, tag="ws")
                    nc.vector.tensor_tensor(
                        out=ws[:], in0=sc[:], in1=ew_t[:],
                        op=mybir.AluOpType.mult)
                    ex = wp.tile([128, T, H], BF16, tag="ex")
                    nc.scalar.activation(
                        out=ex[:], in_=ws[:],
                        func=mybir.ActivationFunctionType.Exp)
                    ctb = wp.tile([128, T, D + H], BF16, tag="ctb")
                    nc.gpsimd.tensor_copy(ctb[:, :, D:D + H], ex[:])
                    nc.vector.tensor_tensor(
                        out=ctb[:, :, 0:D].rearrange("p t (h d) -> p t h d", h=H),
                        in0=kvg[:, :, D:2 * D].rearrange("p t (h d) -> p t h d", h=H),
                        in1=ex[:].rearrange("p t (h o) -> p t h o", o=1)
                            .to_broadcast([128, T, H, HD]),
                        op=mybir.AluOpType.mult)
                    sel = wp.tile([128, T, 128], BF16, tag="sel")
                    sel_eng = nc.gpsimd if sg_i % 2 else nc.vector
                    sel_eng.tensor_tensor(
                        out=sel[:],
                        in0=oid_t[:].rearrange("p (t o) -> p t o", o=1)
                            .to_broadcast([128, T, 128]),
                        in1=iot_t[:].rearrange("p (o n) -> p o n", o=1)
                            .to_broadcast([128, T, 128]),
                        op=mybir.AluOpType.is_equal)

                    psums = {}
                    for b in sg["blocks"]:
                        psums[b] = bp.tile([128, D + H], F32, tag="bps",
                                           name=f"bps{b}")
                    for i, b in enumerate(sg["tile_blocks"]):
                        nc.tensor.matmul(
                            out=psums[b][:],
                            lhsT=sel[:, i, :],
                            rhs=ctb[:, i, :],
                            start=(sg["first"][b] == i),
                            stop=(sg["last"][b] == i))

                    # ---- epilogue, slabbed over the sg's blocks
                    blocks = sg["blocks"]
                    NBk = len(blocks)
                    b0 = blocks[0]
                    zr = pp.tile([128, NBk, H], F32, tag="zr")
                    vals = pp.tile([128, NBk, D], F32, tag="vals")
                    for i, b in enumerate(blocks):
                        nc.vector.tensor_scalar_add(
                            zr[:, i, :], psums[b][:, D:D + H], 1e-16)
                    nc.vector.reciprocal(zr[:], zr[:])
                    for i, b in enumerate(blocks):
                        nc.vector.tensor_tensor(
                            out=vals[:, i, :].rearrange("p (h d) -> p h d", h=H),
                            in0=psums[b][:, 0:D].rearrange("p (h d) -> p h d", h=H),
                            in1=zr[:, i, :].rearrange("p (h o) -> p h o", o=1)
                                .to_broadcast([128, H, HD]),
                            op=mybir.AluOpType.mult)
                    po = op.tile([128, NBk, D], F32, tag="po")
                    for i in range(NBk):
                        pt = tp.tile([D, 128], F32, tag="pt", name=f"pt{i}")
                        nc.tensor.transpose(out=pt[:], in_=vals[:, i, :],
                                            identity=idn_t[:])
                        vT = pp.tile([D, 128], BF16, tag="vT", name=f"vT{i}")
                        nc.vector.tensor_copy(vT[:], pt[:])
                        nc.tensor.matmul(out=po[:, i, :], lhsT=vT[:], rhs=wot_t[:],
                                         start=True, stop=True)
                    # LayerNorm + residual (slab ops over [128, NBk, D])
                    nmu = pp.tile([128, NBk], F32, tag="nmu")
                    nc.vector.tensor_reduce(
                        out=nmu[:], in_=po[:],
                        axis=mybir.AxisListType.X, op=mybir.AluOpType.add)
                    nc.vector.tensor_scalar_mul(nmu[:], nmu[:], -1.0 / D)
                    ct = pp.tile([128, NBk, D], F32, tag="ct")
                    nc.vector.tensor_tensor(
                        out=ct[:], in0=po[:],
                        in1=nmu[:].rearrange("p (b o) -> p b o", o=1)
                            .to_broadcast([128, NBk, D]),
                        op=mybir.AluOpType.add)
                    nc.gpsimd.tensor_tensor(
                        out=ct[:], in0=ct[:],
                        in1=boc_t[:].rearrange("p (o d) -> p o d", o=1)
                            .to_broadcast([128, NBk, D]),
                        op=mybir.AluOpType.add)
                    sq = pp.tile([128, NBk, D], F32, tag="sq")
                    nc.gpsimd.tensor_tensor(
                        out=sq[:], in0=ct[:], in1=ct[:], op=mybir.AluOpType.mult)
                    v1 = pp.tile([128, NBk], F32, tag="v1")
                    nc.vector.tensor_reduce(
                        out=v1[:], in_=sq[:],
                        axis=mybir.AxisListType.X, op=mybir.AluOpType.add)
                    nc.vector.tensor_scalar(
                        out=v1[:], in0=v1[:],
                        scalar1=1.0 / D, scalar2=LN_EPS,
                        op0=mybir.AluOpType.mult, op1=mybir.AluOpType.add)
                    nc.vector.reciprocal(v1[:], v1[:])
                    rstd = pp.tile([128, NBk], F32, tag="rstd")
                    nc.scalar.sqrt(rstd[:], v1[:])
                    xb = pp.tile([128, NBk, D], F32, tag="xb")
                    nc.sync.dma_start(
                        out=xb[:],
                        in_=xpb[b0 * 128:(b0 + NBk) * 128, :]
                            .rearrange("(a p) d -> p a d", p=128))
                    ot = pp.tile([128, NBk, D], F32, tag="ot")
                    nc.vector.tensor_tensor(
                        out=ot[:], in0=ct[:],
                        in1=rstd[:].rearrange("p (b o) -> p b o", o=1)
                            .to_broadcast([128, NBk, D]),
                        op=mybir.AluOpType.mult)
                    nc.gpsimd.tensor_tensor(
                        out=ot[:], in0=ot[:],
                        in1=gam_t[:].rearrange("p (o d) -> p o d", o=1)
                            .to_broadcast([128, NBk, D]),
                        op=mybir.AluOpType.mult)
                    nc.gpsimd.tensor_tensor(
                        out=ot[:], in0=ot[:], in1=xb[:], op=mybir.AluOpType.add)
                    nc.sync.dma_start(
                        out=out[b0 * 128:(b0 + NBk) * 128, :]
                            .rearrange("(a p) d -> p a d", p=128),
                        in_=ot[:])
    return nc


def kernel(x, edge_index, edge_weights, Wq, bq, Wk, bk, Wv, bv, Wo, bo,
           gamma, beta):
    x = np.asarray(x, np.float32)
    edge_index = np.asarray(edge_index)
    edge_weights = np.asarray(edge_weights, np.float32)
    origins = np.asarray(edge_index[0], np.int64)
    dests = np.asarray(edge_index[1], np.int64)

    struct, per_core_raw = _build_structure(origins, dests)
    nc = _build_graph(struct)
    nc.finalize()

    # shared (replicated) host arrays
    xT = np.zeros((D + 1, NT), np.float32)
    xT[:D, :N] = x.T
    xT[D] = 1.0
    xT = xT.astype(BF16_NP)
    wkv = np.zeros((D + 1, 2 * D), np.float32)
    wkv[:D, :D] = np.asarray(Wk, np.float32).T
    wkv[:D, D:] = np.asarray(Wv, np.float32).T
    wkv[D, :D] = np.asarray(bk, np.float32)
    wkv[D, D:] = np.asarray(bv, np.float32)
    wq = np.zeros((D + 1, 2 * D), np.float32)
    wq[:D, :D] = np.asarray(Wq, np.float32).T
    wq[D, :D] = np.asarray(bq, np.float32)
    wkv = wkv.astype(BF16_NP)
    wq = wq.astype(BF16_NP)
    wot = np.ascontiguousarray(np.asarray(Wo, np.float32).T)
    bo = np.asarray(bo, np.float32)
    boc = np.tile((bo - bo.mean())[None, :], (128, 1)).astype(np.float32)
    gam_t = np.tile(np.asarray(gamma, np.float32)[None, :], (128, 1))
    iot = np.tile(np.arange(128, dtype=np.float32)[None, :], (128, 1)
                  ).astype(BF16_NP)
    idn = np.eye(128, dtype=np.float32)

    in_maps = []
    for c in range(NCORES):
        data = _per_core_arrays(struct, per_core_raw[c], edge_weights)
        xTo = np.zeros((D + 1, NB), np.float32)
        xTo[:D, :NOWN] = x[c * NOWN:(c + 1) * NOWN].T
        xTo[D] = 1.0
        xTo = xTo.astype(BF16_NP)
        xpb = np.zeros((NB, D), np.float32)
        xpb[:NOWN] = x[c * NOWN:(c + 1) * NOWN] + np.asarray(beta, np.float32)
        in_maps.append({
            "xT": xT, "xTo": xTo, "wkv": wkv, "wq": wq, "wot": wot,
            "boc": boc, "gam": gam_t, "iot": iot, "idn": idn, "xpb": xpb,
            "oid": data["oid"], "ewp": data["ew"],
            "kvx": data["kvidx"], "qx": data["qidx"],
        })

    res = run_bass_kernel_spmd(nc, in_maps, core_ids=list(range(NCORES)),
                               trace=TRACE)
    global LAST_RESULT
    LAST_RESULT = res
    outs = [np.asarray(res.results[i]["out"])[:NOWN] for i in range(NCORES)]
    return np.concatenate(outs, axis=0).astype(np.float32)


TRACE = False
LAST_RESULT = None


# revision 17
# speedup vs baseline: 1.9691x; 1.0824x over previous
"""Trainium2 Bass kernel for BaseDependentAttentionLayer (GNN message passing).

Strategy (8 NeuronCores, SPMD, no collectives):
  - Nodes sharded by origin: core c owns nodes [c*12500, (c+1)*12500).
  - Every core recomputes the bf16 k|v table for ALL nodes (cheap on PE,
    hides under gather DMA) and q for its own nodes; tables live in DRAM.
  - Edges sharded by origin core, bucketed by (dest-chunk, origin-block),
    padded to 128-edge tiles; per-edge k|v and q fetched with dma_gather
    (int16 indices -> kv table split into 4 chunks of 25088 rows).
  - Segment softmax runs without the max-subtraction pass (shift-invariant;
    values bounded here), so attention reduces to two segment sums that are
    computed with per-tile 0/1 selection-matrix matmuls accumulating into a
    per-128-node-block PSUM tile.  Epilogue (divide, Wo, LayerNorm, residual)
    is fused per block.
"""

import sys

sys.path.insert(0, "/opt/trn_rl_repo")

import numpy as np
import ml_dtypes

import concourse.bass as bass
import concourse.bacc as bacc
import concourse.mybir as mybir
from concourse.tile import TileContext
from concourse.bass_utils import run_bass_kernel_spmd

N = 100000
E = 1600000
D = 64
H = 4
HD = 16
NCORES = 8
NOWN = 12500            # nodes owned per core
NBLK = 98               # 128-node blocks per core
NB = NBLK * 128         # 12544 padded own nodes
NT = 100352             # padded global table rows (= 4 * 25088)
CH = 4                  # dest chunks (int16 gather index limit)
CHROWS = NT // CH       # 25088
GBLK = 3                # node blocks per super-group
LN_EPS = 1e-5
PAD_OID = 200.0         # origin-id sentinel for pad slots (matches no node)

F32 = mybir.dt.float32
BF16 = mybir.dt.bfloat16
I16 = mybir.dt.int16
BF16_NP = ml_dtypes.bfloat16


def _build_structure(origins, dests):
    """Global (core-independent) stream structure + per-core slot data.

    origins/dests: full [E] int arrays.
    Returns (struct, per_core) where struct is identical for all cores.
    """
    owner = origins // NOWN
    per_core_raw = []
    tcnt = np.zeros((NCORES, CH * NBLK), np.int64)
    for c in range(NCORES):
        m = owner == c
        o = (origins[m] - c * NOWN).astype(np.int32)
        d = dests[m].astype(np.int32)
        eids = np.nonzero(m)[0]
        blk = o >> 7
        chunk = d // CHROWS
        cell = chunk * NBLK + blk
        order = np.argsort(cell, kind="stable")
        o, d, eids, cell = o[order], d[order], eids[order], cell[order]
        cnt = np.bincount(cell, minlength=CH * NBLK)
        tcnt[c] = (cnt + 127) // 128
        per_core_raw.append((o, d, eids, cnt))
    cell_tiles = tcnt.max(0)          # [CH*NBLK] tiles per cell, all cores
    cell_tiles = np.maximum(cell_tiles, 1)

    sg_blocks = [list(range(s, min(s + GBLK, NBLK))) for s in range(0, NBLK, GBLK)]
    # stream order: sg -> chunk -> block
    sgs = []
    tile_block = []     # per global tile: block id
    tile_sg = []
    cell_tile_off = np.zeros(CH * NBLK, np.int64)
    t_off = 0
    for blocks in sg_blocks:
        T_c = []
        sg_tiles = []
        for ch in range(CH):
            tc = 0
            for b in blocks:
                cell = ch * NBLK + b
                nt = int(cell_tiles[cell])
                cell_tile_off[cell] = t_off
                tile_block.extend([b] * nt)
                sg_tiles.extend([b] * nt)
                tile_sg.extend([len(sgs)] * nt)
                t_off += nt
                tc += nt
            T_c.append(tc)
        # start/stop flags within the sg
        first = {}
        last = {}
        for i, b in enumerate(sg_tiles):
            if b not in first:
                first[b] = i
            last[b] = i
        sgs.append({
            "blocks": blocks,
            "T_c": T_c,
            "T": sum(T_c),
            "tile_blocks": sg_tiles,
            "first": first,
            "last": last,
            "tile_off": t_off - sum(T_c),
        })
    S_tiles = t_off
    struct = {
        "sgs": sgs,
        "S_tiles": S_tiles,
        "cell_tiles": cell_tiles,
        "cell_tile_off": cell_tile_off,
    }
    return struct, per_core_raw


def _per_core_arrays(struct, core_raw, edge_weights):
    """Build the per-core DRAM-side slot arrays."""
    o, d, eids, cnt = core_raw
    S_tiles = struct["S_tiles"]
    S = S_tiles * 128
    cell_tiles = struct["cell_tiles"]
    cell_tile_off = struct["cell_tile_off"]

    oid = np.full(S, PAD_OID, np.float32)
    kvi = np.zeros(S, np.int16)
    qi = np.zeros(S, np.int16)
    ew4 = np.zeros((S, H), np.float32)

    # place each cell's edges at its slot range
    cell_edge_off = np.zeros(CH * NBLK + 1, np.int64)
    np.cumsum(cnt, out=cell_edge_off[1:])
    for cell in range(CH * NBLK):
        n = int(cnt[cell])
        if n == 0:
            continue
        e0 = cell_edge_off[cell]
        s0 = cell_tile_off[cell] * 128
        ch = cell // NBLK
        sl = slice(s0, s0 + n)
        el = slice(e0, e0 + n)
        oid[sl] = (o[el] & 127).astype(np.float32)
        kvi[sl] = (d[el] - ch * CHROWS).astype(np.int16)
        qi[sl] = o[el].astype(np.int16)
        ew4[sl] = edge_weights[eids[el]] * (HD ** -0.5)

    # tile-major [128, S_tiles(,H)] views (partition = slot % 128)
    oid_t = np.ascontiguousarray(
        oid.reshape(S_tiles, 128).T).astype(BF16_NP)
    ew_t = np.ascontiguousarray(ew4.reshape(S_tiles, 128, H).transpose(1, 0, 2))

    # wrapped int16 index arrays: per-run [16, len/16] replicated to 128 parts
    def wrap(run_vals):
        w = run_vals.reshape(-1, 16).T          # [16, len/16]
        return np.tile(w, (8, 1))               # [128, len/16]

    kvw = np.zeros((128, S_tiles * 8), np.int16)
    qw = np.zeros((128, S_tiles * 8), np.int16)
    for sg in struct["sgs"]:
        t0 = sg["tile_off"]
        qw[:, t0 * 8:(t0 + sg["T"]) * 8] = wrap(qi[t0 * 128:(t0 + sg["T"]) * 128])
        off = t0
        for ch in range(CH):
            tc = sg["T_c"][ch]
            kvw[:, off * 8:(off + tc) * 8] = wrap(kvi[off * 128:(off + tc) * 128])
            off += tc
    return {"oid": oid_t, "ew": ew_t, "kvidx": kvw, "qidx": qw}


def _build_graph(struct, sg_limit=None, dump_tab=False, no_gather=False,
                 gather_only=False):
    nc = bacc.Bacc()
    S_tiles = struct["S_tiles"]

    xT = nc.declare_dram_parameter("xT", [D + 1, NT], BF16, isOutput=False)
    xTo = nc.declare_dram_parameter("xTo", [D + 1, NB], BF16, isOutput=False)
    wkv = nc.declare_dram_parameter("wkv", [D + 1, 2 * D], BF16, isOutput=False)
    wq = nc.declare_dram_parameter("wq", [D + 1, 2 * D], BF16, isOutput=False)
    wot = nc.declare_dram_parameter("wot", [D, D], F32, isOutput=False)
    boc = nc.declare_dram_parameter("boc", [128, D], F32, isOutput=False)
    gam = nc.declare_dram_parameter("gam", [128, D], F32, isOutput=False)
    TMAX = max(sg["T"] for sg in struct["sgs"])
    iot = nc.declare_dram_parameter("iot", [128, 128, TMAX], BF16, isOutput=False)
    idn = nc.declare_dram_parameter("idn", [128, 128], F32, isOutput=False)
    xpb = nc.declare_dram_parameter("xpb", [NB, D], F32, isOutput=False)
    oid = nc.declare_dram_parameter("oid", [128, S_tiles], BF16, isOutput=False)
    ewp = nc.declare_dram_parameter("ewp", [128, S_tiles, H], F32, isOutput=False)
    kvx = nc.declare_dram_parameter("kvx", [128, S_tiles * 8], I16, isOutput=False)
    qx = nc.declare_dram_parameter("qx", [128, S_tiles * 8], I16, isOutput=False)
    out = nc.declare_dram_parameter("out", [NB, D], F32, isOutput=True)

    kv_tab = nc.dram_tensor("kv_tab", [NT, 2 * D], BF16)
    q_tab = nc.dram_tensor("q_tab", [NB, 2 * D], BF16)
    if dump_tab:
        kv_dump = nc.declare_dram_parameter(
            "kv_dump", [1024, 2 * D], BF16, isOutput=True)
        q_dump = nc.declare_dram_parameter(
            "q_dump", [1024, 2 * D], BF16, isOutput=True)
    if gather_only:
        T0 = struct["sgs"][0]["T"]
        g_dump = nc.declare_dram_parameter(
            "g_dump", [128, T0, 2 * D], BF16, isOutput=True)
        g_dump2 = nc.declare_dram_parameter(
            "g_dump2", [128, T0, 2 * D], BF16, isOutput=True)

    with TileContext(nc) as tc:
        with tc.tile_pool(name="const", bufs=1) as cp:
            wkv_t = cp.tile([D + 1, 2 * D], BF16)
            nc.sync.dma_start(out=wkv_t[:], in_=wkv[:])
            wq_t = cp.tile([D + 1, 2 * D], BF16)
            nc.sync.dma_start(out=wq_t[:], in_=wq[:])
            wot_f = cp.tile([D, D], F32)
            nc.sync.dma_start(out=wot_f[:], in_=wot[:])
            wot_t = cp.tile([D, D], BF16)
            nc.vector.tensor_copy(wot_t[:], wot_f[:])
            boc_t = cp.tile([128, D], F32)
            nc.sync.dma_start(out=boc_t[:], in_=boc[:])
            gam_t = cp.tile([128, D], F32)
            nc.sync.dma_start(out=gam_t[:], in_=gam[:])
            iot_t = cp.tile([128, 128, TMAX], BF16)
            nc.sync.dma_start(out=iot_t[:], in_=iot[:])
            idn_t = cp.tile([128, 128], F32)
            nc.sync.dma_start(out=idn_t[:], in_=idn[:])

            # ---- phase 1: build kv table (all nodes) + q table (own nodes)
            SLAB = 16                                          # tiles per slab
            with (
                tc.tile_pool(name="p1sb", bufs=3) as p1,
                tc.tile_pool(name="p1ps", bufs=2, space="PSUM") as p1p,
            ):
                def qkv_slab(src, wt, tab, s, ntile):
                    n0 = s * SLAB * 128
                    xs = p1.tile([D + 1, ntile * 128], BF16, tag="xs")
                    nc.scalar.dma_start(
                        out=xs[:], in_=src[:, n0:n0 + ntile * 128])
                    ps = p1p.tile([128, ntile * 128], F32, tag="ps")
                    for j in range(ntile):
                        nc.tensor.matmul(
                            out=ps[:, j * 128:(j + 1) * 128],
                            lhsT=xs[:, j * 128:(j + 1) * 128],
                            rhs=wt[:],
                            start=True, stop=True)
                    sb = p1.tile([128, ntile, 128], BF16, tag="sb")
                    nc.scalar.copy(
                        sb[:].rearrange("p a d -> p (a d)"),
                        ps[:])
                    nc.sync.dma_start(
                        out=tab[n0:n0 + ntile * 128, :]
                            .rearrange("(a p) d -> p a d", p=128),
                        in_=sb[:])

                for s in range(NT // (SLAB * 128)):            # 49 slabs
                    qkv_slab(xT, wkv_t, kv_tab, s, SLAB)
                nfull_q = NB // (SLAB * 128)                   # 6 slabs
                for s in range(nfull_q):
                    qkv_slab(xTo, wq_t, q_tab, s, SLAB)
                qkv_slab(xTo, wq_t, q_tab, nfull_q,
                         (NB - nfull_q * SLAB * 128) // 128)

            if dump_tab:
                nc.sync.dma_start(out=kv_dump[:], in_=kv_tab[0:1024, :])
                nc.sync.dma_start(out=q_dump[:], in_=q_tab[0:1024, :])
            sgs_run = struct["sgs"] if sg_limit is None else struct["sgs"][:sg_limit]

            # ---- phase 2: edge processing per super-group
            with (
                tc.tile_pool(name="gat", bufs=2) as gp,
                tc.tile_pool(name="met", bufs=2) as mp,
                tc.tile_pool(name="wrk", bufs=2) as wp,
                tc.tile_pool(name="pst", bufs=2) as pp,
                tc.tile_pool(name="bps", bufs=GBLK + 1, space="PSUM") as bp,
                tc.tile_pool(name="tps", bufs=2, space="PSUM") as tp,
                tc.tile_pool(name="ops", bufs=2, space="PSUM") as op,
            ):
                for sg_i, sg in enumerate(sgs_run):
                    T = sg["T"]
                    t0 = sg["tile_off"]
                    kvi_t = mp.tile([128, T * 8], I16, tag="kvi")
                    nc.scalar.dma_start(out=kvi_t[:], in_=kvx[:, t0 * 8:(t0 + T) * 8])
                    qi_t = mp.tile([128, T * 8], I16, tag="qi")
                    nc.scalar.dma_start(out=qi_t[:], in_=qx[:, t0 * 8:(t0 + T) * 8])
                    oid_t = mp.tile([128, T], BF16, tag="oid")
                    nc.sync.dma_start(out=oid_t[:], in_=oid[:, t0:t0 + T])
                    ew_t = mp.tile([128, T, H], F32, tag="ew")
                    nc.sync.dma_start(out=ew_t[:], in_=ewp[:, t0:t0 + T, :])

                    kvg = gp.tile([128, T, 2 * D], BF16, tag="kvg")
                    qg = gp.tile([128, T, 2 * D], BF16, tag="qg")
                    if no_gather:
                        nc.gpsimd.memset(kvg[:], 1.0)
                        nc.gpsimd.memset(qg[:], 1.0)
                    else:
                        off = 0
                        for ch in range(CH):
                            tcn = sg["T_c"][ch]
                            nc.gpsimd.dma_gather(
                                out_ap=kvg[:, off:off + tcn, :],
                                in_ap=kv_tab[ch * CHROWS:(ch + 1) * CHROWS, :],
                                idxs_ap=kvi_t[:, off * 8:(off + tcn) * 8],
                                num_idxs=tcn * 128,
                                num_idxs_reg=tcn * 128,
                                elem_size=2 * D,
                                single_packet=False)
                            off += tcn
                        nc.gpsimd.dma_gather(
                            out_ap=qg[:],
                            in_ap=q_tab[:],
                            idxs_ap=qi_t[:],
                            num_idxs=T * 128,
                            num_idxs_reg=T * 128,
                            elem_size=2 * D,
                            single_packet=False)
                    if gather_only:
                        nc.sync.dma_start(out=g_dump[:], in_=kvg[:])
                        nc.sync.dma_start(out=g_dump2[:], in_=qg[:])
                        continue

                    qk = wp.tile([128, T, D], BF16, tag="qk")
                    nc.vector.tensor_tensor(
                        out=qk[:], in0=qg[:, :, 0:D], in1=kvg[:, :, 0:D],
                        op=mybir.AluOpType.mult)
                    sc = wp.tile([128, T, H], F32, tag="sc")
                    nc.vector.tensor_reduce(
                        out=sc[:],
                        in_=qk[:].rearrange("p t (h d) -> p t h d", h=H),
                        axis=mybir.AxisListType.X, op=mybir.AluOpType.add)
                    ws = wp.tile([128, T, H], F32, tag="ws")
                    nc.vector.tensor_tensor(
                        out=ws[:], in0=sc[:], in1=ew_t[:],
                        op=mybir.AluOpType.mult)
                    ex = wp.tile([128, T, H], BF16, tag="ex")
                    nc.scalar.activation(
                        out=ex[:], in_=ws[:],
                        func=mybir.ActivationFunctionType.Exp)
                    ctb = wp.tile([128, T, D + H], BF16, tag="ctb")
                    nc.vector.tensor_copy(ctb[:, :, D:D + H], ex[:])
                    nc.vector.tensor_tensor(
                        out=ctb[:, :, 0:D].rearrange("p t (e h) -> p t e h", h=H),
                        in0=kvg[:, :, D:2 * D].rearrange("p t (e h) -> p t e h", h=H),
                        in1=ex[:].rearrange("p t (o h) -> p t o h", o=1)
                            .to_broadcast([128, T, HD, H]),
                        op=mybir.AluOpType.mult)
                    sel = wp.tile([128, 128, T], BF16, tag="sel")
                    nc.vector.tensor_tensor(
                        out=sel[:],
                        in0=oid_t[:].rearrange("p (o t) -> p o t", o=1)
                            .to_broadcast([128, 128, T]),
                        in1=iot_t[:, :, 0:T],
                        op=mybir.AluOpType.is_equal)

                    psums = {}
                    for b in sg["blocks"]:
                        psums[b] = bp.tile([128, D + H], F32, tag="bps",
                                           name=f"bps{b}")
                    for i, b in enumerate(sg["tile_blocks"]):
                        nc.tensor.matmul(
                            out=psums[b][:],
                            lhsT=sel[:, :, i],
                            rhs=ctb[:, i, :],
                            start=(sg["first"][b] == i),
                            stop=(sg["last"][b] == i))

                    # ---- epilogue, slabbed over the sg's blocks
                    blocks = sg["blocks"]
                    NBk = len(blocks)
                    b0 = blocks[0]
                    zr = pp.tile([128, NBk, H], F32, tag="zr")
                    vals = pp.tile([128, NBk, D], F32, tag="vals")
                    for i, b in enumerate(blocks):
                        nc.vector.tensor_scalar_add(
                            zr[:, i, :], psums[b][:, D:D + H], 1e-16)
                    nc.vector.reciprocal(zr[:], zr[:])
                    for i, b in enumerate(blocks):
                        nc.vector.tensor_tensor(
                            out=vals[:, i, :].rearrange("p (e h) -> p e h", h=H),
                            in0=psums[b][:, 0:D].rearrange("p (e h) -> p e h", h=H),
                            in1=zr[:, i, :].rearrange("p (o h) -> p o h", o=1)
                                .to_broadcast([128, HD, H]),
                            op=mybir.AluOpType.mult)
                    po = op.tile([128, NBk, D], F32, tag="po")
                    for i in range(NBk):
                        pt = tp.tile([D, 128], F32, tag="pt", name=f"pt{i}")
                        nc.tensor.transpose(out=pt[:], in_=vals[:, i, :],
                                            identity=idn_t[:])
                        vT = pp.tile([D, 128], BF16, tag="vT", name=f"vT{i}")
                        nc.vector.tensor_copy(vT[:], pt[:])
                        nc.tensor.matmul(out=po[:, i, :], lhsT=vT[:], rhs=wot_t[:],
                                         start=True, stop=True)
                    # LayerNorm + residual (slab ops over [128, NBk, D])
                    nmu = pp.tile([128, NBk], F32, tag="nmu")
                    nc.vector.tensor_reduce(
                        out=nmu[:], in_=po[:],
                        axis=mybir.AxisListType.X, op=mybir.AluOpType.add)
                    nc.vector.tensor_scalar_mul(nmu[:], nmu[:], -1.0 / D)
                    ct = pp.tile([128, NBk, D], F32, tag="ct")
                    nc.vector.tensor_tensor(
                        out=ct[:], in0=po[:],
                        in1=nmu[:].rearrange("p (b o) -> p b o", o=1)
                            .to_broadcast([128, NBk, D]),
                        op=mybir.AluOpType.add)
                    nc.gpsimd.tensor_tensor(
                        out=ct[:], in0=ct[:],
                        in1=boc_t[:].rearrange("p (o d) -> p o d", o=1)
                            .to_broadcast([128, NBk, D]),
                        op=mybir.AluOpType.add)
                    sq = pp.tile([128, NBk, D], F32, tag="sq")
                    nc.gpsimd.tensor_tensor(
                        out=sq[:], in0=ct[:], in1=ct[:], op=mybir.AluOpType.mult)
                    v1 = pp.tile([128, NBk], F32, tag="v1")
                    nc.vector.tensor_reduce(
                        out=v1[:], in_=sq[:],
                        axis=mybir.AxisListType.X, op=mybir.AluOpType.add)
                    nc.vector.tensor_scalar(
                        out=v1[:], in0=v1[:],
                        scalar1=1.0 / D, scalar2=LN_EPS,
                        op0=mybir.AluOpType.mult, op1=mybir.AluOpType.add)
                    nc.vector.reciprocal(v1[:], v1[:])
                    rstd = pp.tile([128, NBk], F32, tag="rstd")
                    nc.scalar.sqrt(rstd[:], v1[:])
                    xb = pp.tile([128, NBk, D], F32, tag="xb")
                    nc.sync.dma_start(
                        out=xb[:],
                        in_=xpb[b0 * 128:(b0 + NBk) * 128, :]
                            .rearrange("(a p) d -> p a d", p=128))
                    ot = pp.tile([128, NBk, D], F32, tag="ot")
                    nc.vector.tensor_tensor(
                        out=ot[:], in0=ct[:],
                        in1=rstd[:].rearrange("p (b o) -> p b o", o=1)
                            .to_broadcast([128, NBk, D]),
                        op=mybir.AluOpType.mult)
                    nc.gpsimd.tensor_tensor(
                        out=ot[:], in0=ot[:],
                        in1=gam_t[:].rearrange("p (o d) -> p o d", o=1)
                            .to_broadcast([128, NBk, D]),
                        op=mybir.AluOpType.mult)
                    nc.gpsimd.tensor_tensor(
                        out=ot[:], in0=ot[:], in1=xb[:], op=mybir.AluOpType.add)
                    nc.sync.dma_start(
                        out=out[b0 * 128:(b0 + NBk) * 128, :]
                            .rearrange("(a p) d -> p a d", p=128),
                        in_=ot[:])
    return nc


def kernel(x, edge_index, edge_weights, Wq, bq, Wk, bk, Wv, bv, Wo, bo,
           gamma, beta):
    x = np.asarray(x, np.float32)
    edge_index = np.asarray(edge_index)
    edge_weights = np.asarray(edge_weights, np.float32)
    origins = np.asarray(edge_index[0], np.int64)
    dests = np.asarray(edge_index[1], np.int64)

    struct, per_core_raw = _build_structure(origins, dests)
    nc = _build_graph(struct)
    nc.finalize()

    # shared (replicated) host arrays
    xT = np.zeros((D + 1, NT), np.float32)
    xT[:D, :N] = x.T
    xT[D] = 1.0
    xT = xT.astype(BF16_NP)
    vperm = (np.arange(H)[None, :] * HD + np.arange(HD)[:, None]).ravel()
    wkv = np.zeros((D + 1, 2 * D), np.float32)
    wkv[:D, :D] = np.asarray(Wk, np.float32).T
    wkv[:D, D:] = np.asarray(Wv, np.float32).T[:, vperm]
    wkv[D, :D] = np.asarray(bk, np.float32)
    wkv[D, D:] = np.asarray(bv, np.float32)[vperm]
    wq = np.zeros((D + 1, 2 * D), np.float32)
    wq[:D, :D] = np.asarray(Wq, np.float32).T
    wq[D, :D] = np.asarray(bq, np.float32)
    wkv = wkv.astype(BF16_NP)
    wq = wq.astype(BF16_NP)
    wot = np.ascontiguousarray(np.asarray(Wo, np.float32).T[vperm, :])
    bo = np.asarray(bo, np.float32)
    boc = np.tile((bo - bo.mean())[None, :], (128, 1)).astype(np.float32)
    gam_t = np.tile(np.asarray(gamma, np.float32)[None, :], (128, 1))
    TMAX = max(sg["T"] for sg in struct["sgs"])
    iot = np.tile(np.arange(128, dtype=np.float32)[None, :, None],
                  (128, 1, TMAX)).astype(BF16_NP)
    idn = np.eye(128, dtype=np.float32)

    in_maps = []
    for c in range(NCORES):
        data = _per_core_arrays(struct, per_core_raw[c], edge_weights)
        xTo = np.zeros((D + 1, NB), np.float32)
        xTo[:D, :NOWN] = x[c * NOWN:(c + 1) * NOWN].T
        xTo[D] = 1.0
        xTo = xTo.astype(BF16_NP)
        xpb = np.zeros((NB, D), np.float32)
        xpb[:NOWN] = x[c * NOWN:(c + 1) * NOWN] + np.asarray(beta, np.float32)
        in_maps.append({
            "xT": xT, "xTo": xTo, "wkv": wkv, "wq": wq, "wot": wot,
            "boc": boc, "gam": gam_t, "iot": iot, "idn": idn, "xpb": xpb,
            "oid": data["oid"], "ewp": data["ew"],
            "kvx": data["kvidx"], "qx": data["qidx"],
        })

    res = run_bass_kernel_spmd(nc, in_maps, core_ids=list(range(NCORES)),
                               trace=TRACE)
    global LAST_RESULT
    LAST_RESULT = res
    outs = [np.asarray(res.results[i]["out"])[:NOWN] for i in range(NCORES)]
    return np.concatenate(outs, axis=0).astype(np.float32)


TRACE = False
LAST_RESULT = None


# revision 18
# speedup vs baseline: 1.9886x; 1.0099x over previous
"""Trainium2 Bass kernel for BaseDependentAttentionLayer (GNN message passing).

Strategy (8 NeuronCores, SPMD, no collectives):
  - Nodes sharded by origin: core c owns nodes [c*12500, (c+1)*12500).
  - Every core recomputes the bf16 k|v table for ALL nodes (cheap on PE,
    hides under gather DMA) and q for its own nodes; tables live in DRAM.
  - Edges sharded by origin core, bucketed by (dest-chunk, origin-block),
    padded to 128-edge tiles; per-edge k|v and q fetched with dma_gather
    (int16 indices -> kv table split into 4 chunks of 25088 rows).
  - Segment softmax runs without the max-subtraction pass (shift-invariant;
    values bounded here), so attention reduces to two segment sums that are
    computed with per-tile 0/1 selection-matrix matmuls accumulating into a
    per-128-node-block PSUM tile.  Epilogue (divide, Wo, LayerNorm, residual)
    is fused per block.
"""

import sys

sys.path.insert(0, "/opt/trn_rl_repo")

import numpy as np
import ml_dtypes

import concourse.bass as bass
import concourse.bacc as bacc
import concourse.mybir as mybir
from concourse.tile import TileContext
from concourse.bass_utils import run_bass_kernel_spmd

N = 100000
E = 1600000
D = 64
H = 4
HD = 16
NCORES = 8
NOWN = 12500            # nodes owned per core
NBLK = 98               # 128-node blocks per core
NB = NBLK * 128         # 12544 padded own nodes
NT = 100352             # padded global table rows (= 4 * 25088)
CH = 4                  # dest chunks (int16 gather index limit)
CHROWS = NT // CH       # 25088
GBLK = 4                # node blocks per super-group
LN_EPS = 1e-5
PAD_OID = 200.0         # origin-id sentinel for pad slots (matches no node)

F32 = mybir.dt.float32
BF16 = mybir.dt.bfloat16
I16 = mybir.dt.int16
BF16_NP = ml_dtypes.bfloat16


def _build_structure(origins, dests):
    """Global (core-independent) stream structure + per-core slot data.

    origins/dests: full [E] int arrays.
    Returns (struct, per_core) where struct is identical for all cores.
    """
    owner = origins // NOWN
    per_core_raw = []
    tcnt = np.zeros((NCORES, CH * NBLK), np.int64)
    for c in range(NCORES):
        m = owner == c
        o = (origins[m] - c * NOWN).astype(np.int32)
        d = dests[m].astype(np.int32)
        eids = np.nonzero(m)[0]
        blk = o >> 7
        chunk = d // CHROWS
        cell = chunk * NBLK + blk
        order = np.argsort(cell, kind="stable")
        o, d, eids, cell = o[order], d[order], eids[order], cell[order]
        cnt = np.bincount(cell, minlength=CH * NBLK)
        tcnt[c] = (cnt + 127) // 128
        per_core_raw.append((o, d, eids, cnt))
    cell_tiles = tcnt.max(0)          # [CH*NBLK] tiles per cell, all cores
    cell_tiles = np.maximum(cell_tiles, 1)

    sg_blocks = [list(range(s, min(s + GBLK, NBLK))) for s in range(0, NBLK, GBLK)]
    # stream order: sg -> chunk -> block
    sgs = []
    tile_block = []     # per global tile: block id
    tile_sg = []
    cell_tile_off = np.zeros(CH * NBLK, np.int64)
    t_off = 0
    for blocks in sg_blocks:
        T_c = []
        sg_tiles = []
        for ch in range(CH):
            tc = 0
            for b in blocks:
                cell = ch * NBLK + b
                nt = int(cell_tiles[cell])
                cell_tile_off[cell] = t_off
                tile_block.extend([b] * nt)
                sg_tiles.extend([b] * nt)
                tile_sg.extend([len(sgs)] * nt)
                t_off += nt
                tc += nt
            T_c.append(tc)
        # start/stop flags within the sg
        first = {}
        last = {}
        for i, b in enumerate(sg_tiles):
            if b not in first:
                first[b] = i
            last[b] = i
        sgs.append({
            "blocks": blocks,
            "T_c": T_c,
            "T": sum(T_c),
            "tile_blocks": sg_tiles,
            "first": first,
            "last": last,
            "tile_off": t_off - sum(T_c),
        })
    S_tiles = t_off
    struct = {
        "sgs": sgs,
        "S_tiles": S_tiles,
        "cell_tiles": cell_tiles,
        "cell_tile_off": cell_tile_off,
    }
    return struct, per_core_raw


def _per_core_arrays(struct, core_raw, edge_weights):
    """Build the per-core DRAM-side slot arrays."""
    o, d, eids, cnt = core_raw
    S_tiles = struct["S_tiles"]
    S = S_tiles * 128
    cell_tiles = struct["cell_tiles"]
    cell_tile_off = struct["cell_tile_off"]

    oid = np.full(S, PAD_OID, np.float32)
    kvi = np.zeros(S, np.int16)
    qi = np.zeros(S, np.int16)
    ew4 = np.zeros((S, H), np.float32)

    # place each cell's edges at its slot range
    cell_edge_off = np.zeros(CH * NBLK + 1, np.int64)
    np.cumsum(cnt, out=cell_edge_off[1:])
    for cell in range(CH * NBLK):
        n = int(cnt[cell])
        if n == 0:
            continue
        e0 = cell_edge_off[cell]
        s0 = cell_tile_off[cell] * 128
        ch = cell // NBLK
        sl = slice(s0, s0 + n)
        el = slice(e0, e0 + n)
        oid[sl] = (o[el] & 127).astype(np.float32)
        kvi[sl] = (d[el] - ch * CHROWS).astype(np.int16)
        qi[sl] = o[el].astype(np.int16)
        ew4[sl] = edge_weights[eids[el]] * (HD ** -0.5)

    # tile-major [128, S_tiles(,H)] views (partition = slot % 128)
    oid_t = np.ascontiguousarray(
        oid.reshape(S_tiles, 128).T).astype(BF16_NP)
    ew_t = np.ascontiguousarray(ew4.reshape(S_tiles, 128, H).transpose(1, 0, 2))

    # wrapped int16 index arrays: per-run [16, len/16] replicated to 128 parts
    def wrap(run_vals):
        w = run_vals.reshape(-1, 16).T          # [16, len/16]
        return np.tile(w, (8, 1))               # [128, len/16]

    kvw = np.zeros((128, S_tiles * 8), np.int16)
    qw = np.zeros((128, S_tiles * 8), np.int16)
    for sg in struct["sgs"]:
        t0 = sg["tile_off"]
        qw[:, t0 * 8:(t0 + sg["T"]) * 8] = wrap(qi[t0 * 128:(t0 + sg["T"]) * 128])
        off = t0
        for ch in range(CH):
            tc = sg["T_c"][ch]
            kvw[:, off * 8:(off + tc) * 8] = wrap(kvi[off * 128:(off + tc) * 128])
            off += tc
    return {"oid": oid_t, "ew": ew_t, "kvidx": kvw, "qidx": qw}


def _build_graph(struct, sg_limit=None, dump_tab=False, no_gather=False,
                 gather_only=False):
    nc = bacc.Bacc()
    S_tiles = struct["S_tiles"]

    xT = nc.declare_dram_parameter("xT", [D + 1, NT], BF16, isOutput=False)
    xTo = nc.declare_dram_parameter("xTo", [D + 1, NB], BF16, isOutput=False)
    wkv = nc.declare_dram_parameter("wkv", [D + 1, 2 * D], BF16, isOutput=False)
    wq = nc.declare_dram_parameter("wq", [D + 1, 2 * D], BF16, isOutput=False)
    wot = nc.declare_dram_parameter("wot", [D, D], F32, isOutput=False)
    boc = nc.declare_dram_parameter("boc", [128, D], F32, isOutput=False)
    gam = nc.declare_dram_parameter("gam", [128, D], F32, isOutput=False)
    TMAX = max(sg["T"] for sg in struct["sgs"])
    iot = nc.declare_dram_parameter("iot", [128, 128, TMAX], BF16, isOutput=False)
    idn = nc.declare_dram_parameter("idn", [128, 128], F32, isOutput=False)
    xpb = nc.declare_dram_parameter("xpb", [NB, D], F32, isOutput=False)
    oid = nc.declare_dram_parameter("oid", [128, S_tiles], BF16, isOutput=False)
    ewp = nc.declare_dram_parameter("ewp", [128, S_tiles, H], F32, isOutput=False)
    kvx = nc.declare_dram_parameter("kvx", [128, S_tiles * 8], I16, isOutput=False)
    qx = nc.declare_dram_parameter("qx", [128, S_tiles * 8], I16, isOutput=False)
    out = nc.declare_dram_parameter("out", [NB, D], F32, isOutput=True)

    kv_tab = nc.dram_tensor("kv_tab", [NT, 2 * D], BF16)
    q_tab = nc.dram_tensor("q_tab", [NB, 2 * D], BF16)
    if dump_tab:
        kv_dump = nc.declare_dram_parameter(
            "kv_dump", [1024, 2 * D], BF16, isOutput=True)
        q_dump = nc.declare_dram_parameter(
            "q_dump", [1024, 2 * D], BF16, isOutput=True)
    if gather_only:
        T0 = struct["sgs"][0]["T"]
        g_dump = nc.declare_dram_parameter(
            "g_dump", [128, T0, 2 * D], BF16, isOutput=True)
        g_dump2 = nc.declare_dram_parameter(
            "g_dump2", [128, T0, 2 * D], BF16, isOutput=True)

    with TileContext(nc) as tc:
        with tc.tile_pool(name="const", bufs=1) as cp:
            wkv_t = cp.tile([D + 1, 2 * D], BF16)
            nc.sync.dma_start(out=wkv_t[:], in_=wkv[:])
            wq_t = cp.tile([D + 1, 2 * D], BF16)
            nc.sync.dma_start(out=wq_t[:], in_=wq[:])
            wot_f = cp.tile([D, D], F32)
            nc.sync.dma_start(out=wot_f[:], in_=wot[:])
            wot_t = cp.tile([D, D], BF16)
            nc.vector.tensor_copy(wot_t[:], wot_f[:])
            boc_t = cp.tile([128, D], F32)
            nc.sync.dma_start(out=boc_t[:], in_=boc[:])
            gam_t = cp.tile([128, D], F32)
            nc.sync.dma_start(out=gam_t[:], in_=gam[:])
            iot_t = cp.tile([128, 128, TMAX], BF16)
            nc.sync.dma_start(out=iot_t[:], in_=iot[:])
            idn_t = cp.tile([128, 128], F32)
            nc.sync.dma_start(out=idn_t[:], in_=idn[:])

            # ---- phase 1: build kv table (all nodes) + q table (own nodes)
            SLAB = 16                                          # tiles per slab
            with (
                tc.tile_pool(name="p1sb", bufs=3) as p1,
                tc.tile_pool(name="p1ps", bufs=2, space="PSUM") as p1p,
            ):
                def qkv_slab(src, wt, tab, s, ntile):
                    n0 = s * SLAB * 128
                    xs = p1.tile([D + 1, ntile * 128], BF16, tag="xs")
                    nc.scalar.dma_start(
                        out=xs[:], in_=src[:, n0:n0 + ntile * 128])
                    ps = p1p.tile([128, ntile * 128], F32, tag="ps")
                    for j in range(ntile):
                        nc.tensor.matmul(
                            out=ps[:, j * 128:(j + 1) * 128],
                            lhsT=xs[:, j * 128:(j + 1) * 128],
                            rhs=wt[:],
                            start=True, stop=True)
                    sb = p1.tile([128, ntile, 128], BF16, tag="sb")
                    nc.scalar.copy(
                        sb[:].rearrange("p a d -> p (a d)"),
                        ps[:])
                    nc.sync.dma_start(
                        out=tab[n0:n0 + ntile * 128, :]
                            .rearrange("(a p) d -> p a d", p=128),
                        in_=sb[:])

                for s in range(NT // (SLAB * 128)):            # 49 slabs
                    qkv_slab(xT, wkv_t, kv_tab, s, SLAB)
                nfull_q = NB // (SLAB * 128)                   # 6 slabs
                for s in range(nfull_q):
                    qkv_slab(xTo, wq_t, q_tab, s, SLAB)
                qkv_slab(xTo, wq_t, q_tab, nfull_q,
                         (NB - nfull_q * SLAB * 128) // 128)

            if dump_tab:
                nc.sync.dma_start(out=kv_dump[:], in_=kv_tab[0:1024, :])
                nc.sync.dma_start(out=q_dump[:], in_=q_tab[0:1024, :])
            sgs_run = struct["sgs"] if sg_limit is None else struct["sgs"][:sg_limit]

            # ---- phase 2: edge processing per super-group
            with (
                tc.tile_pool(name="gat", bufs=2) as gp,
                tc.tile_pool(name="met", bufs=2) as mp,
                tc.tile_pool(name="wrk", bufs=2) as wp,
                tc.tile_pool(name="pst", bufs=2) as pp,
                tc.tile_pool(name="bps", bufs=GBLK, space="PSUM") as bp,
                tc.tile_pool(name="tps", bufs=2, space="PSUM") as tp,
                tc.tile_pool(name="ops", bufs=2, space="PSUM") as op,
            ):
                for sg_i, sg in enumerate(sgs_run):
                    T = sg["T"]
                    t0 = sg["tile_off"]
                    kvi_t = mp.tile([128, T * 8], I16, tag="kvi")
                    nc.scalar.dma_start(out=kvi_t[:], in_=kvx[:, t0 * 8:(t0 + T) * 8])
                    qi_t = mp.tile([128, T * 8], I16, tag="qi")
                    nc.scalar.dma_start(out=qi_t[:], in_=qx[:, t0 * 8:(t0 + T) * 8])
                    oid_t = mp.tile([128, T], BF16, tag="oid")
                    nc.sync.dma_start(out=oid_t[:], in_=oid[:, t0:t0 + T])
                    ew_t = mp.tile([128, T, H], F32, tag="ew")
                    nc.sync.dma_start(out=ew_t[:], in_=ewp[:, t0:t0 + T, :])

                    kvg = gp.tile([128, T, 2 * D], BF16, tag="kvg")
                    qg = gp.tile([128, T, 2 * D], BF16, tag="qg")
                    if no_gather:
                        nc.gpsimd.memset(kvg[:], 1.0)
                        nc.gpsimd.memset(qg[:], 1.0)
                    else:
                        off = 0
                        for ch in range(CH):
                            tcn = sg["T_c"][ch]
                            nc.gpsimd.dma_gather(
                                out_ap=kvg[:, off:off + tcn, :],
                                in_ap=kv_tab[ch * CHROWS:(ch + 1) * CHROWS, :],
                                idxs_ap=kvi_t[:, off * 8:(off + tcn) * 8],
                                num_idxs=tcn * 128,
                                num_idxs_reg=tcn * 128,
                                elem_size=2 * D,
                                single_packet=False)
                            off += tcn
                        nc.gpsimd.dma_gather(
                            out_ap=qg[:],
                            in_ap=q_tab[:],
                            idxs_ap=qi_t[:],
                            num_idxs=T * 128,
                            num_idxs_reg=T * 128,
                            elem_size=2 * D,
                            single_packet=False)
                    if gather_only:
                        nc.sync.dma_start(out=g_dump[:], in_=kvg[:])
                        nc.sync.dma_start(out=g_dump2[:], in_=qg[:])
                        continue

                    ctb = wp.tile([128, T, D + H], BF16, tag="ctb")
                    nc.vector.tensor_tensor(
                        out=ctb[:, :, 0:D], in0=qg[:, :, 0:D], in1=kvg[:, :, 0:D],
                        op=mybir.AluOpType.mult)
                    sc = wp.tile([128, T, H], F32, tag="sc")
                    nc.vector.tensor_reduce(
                        out=sc[:],
                        in_=ctb[:, :, 0:D].rearrange("p t (h d) -> p t h d", h=H),
                        axis=mybir.AxisListType.X, op=mybir.AluOpType.add)
                    ws = wp.tile([128, T, H], F32, tag="ws")
                    nc.vector.tensor_tensor(
                        out=ws[:], in0=sc[:], in1=ew_t[:],
                        op=mybir.AluOpType.mult)
                    ex = wp.tile([128, T, H], BF16, tag="ex")
                    nc.scalar.activation(
                        out=ex[:], in_=ws[:],
                        func=mybir.ActivationFunctionType.Exp)
                    nc.vector.tensor_copy(ctb[:, :, D:D + H], ex[:])
                    nc.vector.tensor_tensor(
                        out=ctb[:, :, 0:D].rearrange("p t (e h) -> p t e h", h=H),
                        in0=kvg[:, :, D:2 * D].rearrange("p t (e h) -> p t e h", h=H),
                        in1=ex[:].rearrange("p t (o h) -> p t o h", o=1)
                            .to_broadcast([128, T, HD, H]),
                        op=mybir.AluOpType.mult)
                    sel = wp.tile([128, 128, T], BF16, tag="sel")
                    nc.vector.tensor_tensor(
                        out=sel[:],
                        in0=oid_t[:].rearrange("p (o t) -> p o t", o=1)
                            .to_broadcast([128, 128, T]),
                        in1=iot_t[:, :, 0:T],
                        op=mybir.AluOpType.is_equal)

                    psums = {}
                    for b in sg["blocks"]:
                        psums[b] = bp.tile([128, D + H], F32, tag="bps",
                                           name=f"bps{b}")
                    for i, b in enumerate(sg["tile_blocks"]):
                        nc.tensor.matmul(
                            out=psums[b][:],
                            lhsT=sel[:, :, i],
                            rhs=ctb[:, i, :],
                            start=(sg["first"][b] == i),
                            stop=(sg["last"][b] == i))

                    # ---- epilogue, slabbed over the sg's blocks
                    blocks = sg["blocks"]
                    NBk = len(blocks)
                    b0 = blocks[0]
                    zr = pp.tile([128, NBk, H], F32, tag="zr")
                    vals = pp.tile([128, NBk, D], F32, tag="vals")
                    for i, b in enumerate(blocks):
                        nc.vector.tensor_scalar_add(
                            zr[:, i, :], psums[b][:, D:D + H], 1e-16)
                    nc.vector.reciprocal(zr[:], zr[:])
                    for i, b in enumerate(blocks):
                        nc.vector.tensor_tensor(
                            out=vals[:, i, :].rearrange("p (e h) -> p e h", h=H),
                            in0=psums[b][:, 0:D].rearrange("p (e h) -> p e h", h=H),
                            in1=zr[:, i, :].rearrange("p (o h) -> p o h", o=1)
                                .to_broadcast([128, HD, H]),
                            op=mybir.AluOpType.mult)
                    po = op.tile([128, NBk, D], F32, tag="po")
                    for i in range(NBk):
                        pt = tp.tile([D, 128], F32, tag="pt", name=f"pt{i}")
                        nc.tensor.transpose(out=pt[:], in_=vals[:, i, :],
                                            identity=idn_t[:])
                        vT = pp.tile([D, 128], BF16, tag="vT", name=f"vT{i}")
                        nc.vector.tensor_copy(vT[:], pt[:])
                        nc.tensor.matmul(out=po[:, i, :], lhsT=vT[:], rhs=wot_t[:],
                                         start=True, stop=True)
                    # LayerNorm + residual (slab ops over [128, NBk, D])
                    nmu = pp.tile([128, NBk], F32, tag="nmu")
                    nc.vector.tensor_reduce(
                        out=nmu[:], in_=po[:],
                        axis=mybir.AxisListType.X, op=mybir.AluOpType.add)
                    nc.vector.tensor_scalar_mul(nmu[:], nmu[:], -1.0 / D)
                    ct = pp.tile([128, NBk, D], F32, tag="ct")
                    nc.vector.tensor_tensor(
                        out=ct[:], in0=po[:],
                        in1=nmu[:].rearrange("p (b o) -> p b o", o=1)
                            .to_broadcast([128, NBk, D]),
                        op=mybir.AluOpType.add)
                    nc.gpsimd.tensor_tensor(
                        out=ct[:], in0=ct[:],
                        in1=boc_t[:].rearrange("p (o d) -> p o d", o=1)
                            .to_broadcast([128, NBk, D]),
                        op=mybir.AluOpType.add)
                    sq = pp.tile([128, NBk, D], F32, tag="sq")
                    nc.gpsimd.tensor_tensor(
                        out=sq[:], in0=ct[:], in1=ct[:], op=mybir.AluOpType.mult)
                    v1 = pp.tile([128, NBk], F32, tag="v1")
                    nc.vector.tensor_reduce(
                        out=v1[:], in_=sq[:],
                        axis=mybir.AxisListType.X, op=mybir.AluOpType.add)
                    nc.vector.tensor_scalar(
                        out=v1[:], in0=v1[:],
                        scalar1=1.0 / D, scalar2=LN_EPS,
                        op0=mybir.AluOpType.mult, op1=mybir.AluOpType.add)
                    nc.vector.reciprocal(v1[:], v1[:])
                    rstd = pp.tile([128, NBk], F32, tag="rstd")
                    nc.scalar.sqrt(rstd[:], v1[:])
                    xb = pp.tile([128, NBk, D], F32, tag="xb")
                    nc.sync.dma_start(
                        out=xb[:],
                        in_=xpb[b0 * 128:(b0 + NBk) * 128, :]
                            .rearrange("(a p) d -> p a d", p=128))
                    ot = pp.tile([128, NBk, D], F32, tag="ot")
                    nc.vector.tensor_tensor(
                        out=ot[:], in0=ct[:],
                        in1=rstd[:].rearrange("p (b o) -> p b o", o=1)
                            .to_broadcast([128, NBk, D]),
                        op=mybir.AluOpType.mult)
                    nc.gpsimd.tensor_tensor(
                        out=ot[:], in0=ot[:],
                        in1=gam_t[:].rearrange("p (o d) -> p o d", o=1)
                            .to_broadcast([128, NBk, D]),
                        op=mybir.AluOpType.mult)
                    nc.gpsimd.tensor_tensor(
                        out=ot[:], in0=ot[:], in1=xb[:], op=mybir.AluOpType.add)
                    nc.sync.dma_start(
                        out=out[b0 * 128:(b0 + NBk) * 128, :]
                            .rearrange("(a p) d -> p a d", p=128),
                        in_=ot[:])
    return nc


def kernel(x, edge_index, edge_weights, Wq, bq, Wk, bk, Wv, bv, Wo, bo,
           gamma, beta):
    x = np.asarray(x, np.float32)
    edge_index = np.asarray(edge_index)
    edge_weights = np.asarray(edge_weights, np.float32)
    origins = np.asarray(edge_index[0], np.int64)
    dests = np.asarray(edge_index[1], np.int64)

    struct, per_core_raw = _build_structure(origins, dests)
    nc = _build_graph(struct)
    nc.finalize()

    # shared (replicated) host arrays
    xT = np.zeros((D + 1, NT), np.float32)
    xT[:D, :N] = x.T
    xT[D] = 1.0
    xT = xT.astype(BF16_NP)
    vperm = (np.arange(H)[None, :] * HD + np.arange(HD)[:, None]).ravel()
    wkv = np.zeros((D + 1, 2 * D), np.float32)
    wkv[:D, :D] = np.asarray(Wk, np.float32).T
    wkv[:D, D:] = np.asarray(Wv, np.float32).T[:, vperm]
    wkv[D, :D] = np.asarray(bk, np.float32)
    wkv[D, D:] = np.asarray(bv, np.float32)[vperm]
    wq = np.zeros((D + 1, 2 * D), np.float32)
    wq[:D, :D] = np.asarray(Wq, np.float32).T
    wq[D, :D] = np.asarray(bq, np.float32)
    wkv = wkv.astype(BF16_NP)
    wq = wq.astype(BF16_NP)
    wot = np.ascontiguousarray(np.asarray(Wo, np.float32).T[vperm, :])
    bo = np.asarray(bo, np.float32)
    boc = np.tile((bo - bo.mean())[None, :], (128, 1)).astype(np.float32)
    gam_t = np.tile(np.asarray(gamma, np.float32)[None, :], (128, 1))
    TMAX = max(sg["T"] for sg in struct["sgs"])
    iot = np.tile(np.arange(128, dtype=np.float32)[None, :, None],
                  (128, 1, TMAX)).astype(BF16_NP)
    idn = np.eye(128, dtype=np.float32)

    in_maps = []
    for c in range(NCORES):
        data = _per_core_arrays(struct, per_core_raw[c], edge_weights)
        xTo = np.zeros((D + 1, NB), np.float32)
        xTo[:D, :NOWN] = x[c * NOWN:(c + 1) * NOWN].T
        xTo[D] = 1.0
        xTo = xTo.astype(BF16_NP)
        xpb = np.zeros((NB, D), np.float32)
        xpb[:NOWN] = x[c * NOWN:(c + 1) * NOWN] + np.asarray(beta, np.float32)
        in_maps.append({
            "xT": xT, "xTo": xTo, "wkv": wkv, "wq": wq, "wot": wot,
            "boc": boc, "gam": gam_t, "iot": iot, "idn": idn, "xpb": xpb,
            "oid": data["oid"], "ewp": data["ew"],
            "kvx": data["kvidx"], "qx": data["qidx"],
        })

    res = run_bass_kernel_spmd(nc, in_maps, core_ids=list(range(NCORES)),
                               trace=TRACE)
    global LAST_RESULT
    LAST_RESULT = res
    outs = [np.asarray(res.results[i]["out"])[:NOWN] for i in range(NCORES)]
    return np.concatenate(outs, axis=0).astype(np.float32)


TRACE = False
LAST_RESULT = None
